# revision 24
# baseline (speedup 1.0000x reference)
"""Trainium2 Bass kernel for nn_DiscretisedBNF (histogram binning MLP).

Math: the reference's per-bin CDF sum telescopes exactly, so

    out = 0.5*(1 + erf(arg)),  arg = (0.875 - mu_x) * inv
    mu_x = mu/gamma - s*mu_eps,  inv = exp(-ln_sigma_eps)/(s*sqrt(2))

Rearranged for the chip with fp8 scaling (SX*SW1 = 2048 for mm1,
SH*SW2 = 1024 for mm2; PSUM carries 1024x the true nn outputs):

    arg = (psA + mu*qm + qa) * E
    qm  = -1024/(gamma*s), qa = 896/s            (per batch row, fp32)
    E   = exp(-psB/1024 - ln(1024*sqrt(2)))  ~=  sigmoid(-psB/1024 - C1)
    out_dram = erf(arg)      (host applies 0.5*x + 0.5)

The sigmoid stand-in for exp keeps every activation (Prelu for the MLP's
leaky relu, Sigmoid, Erf) inside the single `sigmoid_and_others` ACT
table set -- one table load for the whole kernel.

Both matmuls run fp8e4 DoubleRow (2 weights/cell).  The b2 bias rides on
hidden unit 1023: W1 col 1023 is zeroed, b1[1023] forced to 1, and W2
row 1023 replaced by b2.

DMA schedule: every DRAM tensor is PARTITION-MAJOR (2-16KB contiguous
per partition -> ~26GB/s per DMA engine, ~420GB/s aggregate over the 16
shared engines).  Everything is fully SBUF-resident, issued up front in
need order -- DMA issue on an engine costs ~0.6us per instruction, so
xT leads the SWDGE queue and the small constants are merged into two
tensors and issued behind the first w2 chunk:
  - gpsimd SWDGE:   xT p0/p1/p2, w2 j0, consts, w2 j1, mun, w2 j2..j7
  - sync/scalar HWDGE: W1 in 9 groups (1,1,2,...) alternating rings
  - sync HWDGE:     out writes as each j-group finishes
a2 = mu*qm+qa runs on gpsimd (idle after its DMA issues) to keep DVE
under mm2's 216ns/matmul streaming rate.
Sharding: pure data parallel -- batch dim (2048) split 256 rows/core.
"""

import numpy as np
from contextlib import ExitStack

import concourse.bass as bass
import concourse.mybir as mybir
from concourse.tile import TileContext
from concourse.tile_rust import add_dep_helper
from concourse.bass_utils import run_bass_kernel_spmd

B, D, H = 2048, 4096, 1024
NCORES = 8
BS = B // NCORES            # 256 batch rows per core
KP1 = 16                    # mm1 DoubleRow pairs over mu rows (4096)
W1_PARTS = [1, 1, 2, 2, 2, 2, 2, 2, 2]   # 9 W1 groups (pairs per group)
XT_PARTS = [4, 6, 6]                     # xT delivery parts (pairs)
KC2 = H // 128              # 8 h chunks of 128
KP2 = KC2 // 2              # 4 mm2 DoubleRow pairs
NJ = D // 512               # 8 output column groups of 512
ALPHA = 0.01                # torch nn.LeakyReLU default
SX, SW1, SH, SW2 = 16.0, 128.0, 16.0, 64.0
SCALE2 = SH * SW2           # 1024: psum = SCALE2 * (h @ W2 + b2)
C1 = float(np.log(SCALE2 * np.sqrt(2.0)))
WARM_N = 0   # warmup matmuls burn the HAM power budget -> mid-mm2 throttle

F8 = mybir.dt.float8e4
F16 = mybir.dt.float16
BF16 = mybir.dt.bfloat16
F32 = mybir.dt.float32
AF = mybir.ActivationFunctionType
OP = mybir.AluOpType
DR = mybir.MatmulPerfMode.DoubleRow


def split_multi_waits(nc):
    """This container's walrus accepts at most ONE sync-wait per instruction
    (setupSyncWait: 'Too many sync wait commands').  Split any instruction
    carrying N>1 waits into N-1 single-wait NoOps on the same engine placed
    immediately before it."""
    cnt = 0
    sync_info_cls = None
    for f in nc.m.functions:
        for bb in f.blocks:
            out = []
            changed = False
            for inst in bb.instructions:
                si = inst.sync_info
                waits = list(si.on_wait) if si and si.on_wait else []
                if len(waits) > 1:
                    if sync_info_cls is None:
                        sync_info_cls = type(si)
                    for w in waits[:-1]:
                        nop = mybir.InstNoOp(name=f"waitsplit_{cnt}", ins=[], outs=[])
                        cnt += 1
                        nop.engine = inst.engine
                        nop.sync_info = sync_info_cls(on_wait=[w], on_update=[])
                        out.append(nop)
                    si.on_wait = waits[-1:]
                    changed = True
                out.append(inst)
            if changed:
                bb.instructions = out
    return cnt


def _lean_drain_and_barrier(self, tick_clock, wait_clock):
    """Replacement for TileContext._drain_and_barrier: drain + ONE barrier,
    skipping the semaphore-clear butterfly (the Bass preamble re-clears all
    kernel semaphores at the start of each execution anyway)."""
    import concourse.tile as tile_mod

    nc = self.nc
    drain_inst = nc.sync.drain()
    wait_clock.add_sem_waits(
        drain_inst.ins, tile_mod.ScopedClock({None: tick_clock.global_clock})
    )
    popped = nc._tile_sem_poison_stack.pop()
    assert popped is self._sem_poison


def _build():
    nc = bass.Bass()
    orig_drain = TileContext._drain_and_barrier
    TileContext._drain_and_barrier = _lean_drain_and_barrier
    try:
        _build_body(nc)
    finally:
        TileContext._drain_and_barrier = orig_drain

    split_multi_waits(nc)
    return nc


def _build_body(nc):
    # all partition-major: per-partition lines are contiguous and big
    xT = nc.dram_tensor("xT", [128, KP1, 2, BS], F8, kind="ExternalInput")
    w1 = nc.dram_tensor("w1", [128, KP1, 2, H], F8, kind="ExternalInput")
    w2 = nc.dram_tensor("w2", [128, NJ, KP2, 2, 2, 512], F8, kind="ExternalInput")
    mun = nc.dram_tensor("mun", [128, NJ, 2, 512], F16, kind="ExternalInput")
    cst8 = nc.dram_tensor("cst8", [1, BS + H], F8, kind="ExternalInput")
    cstf = nc.dram_tensor("cstf", [128, KC2 + 4], F32, kind="ExternalInput")
    outd = nc.dram_tensor("out", [128, NJ, 2, 512], F16, kind="ExternalOutput")

    with TileContext(nc) as tc, ExitStack() as ctx:
        const = ctx.enter_context(tc.tile_pool(name="const", bufs=1))
        xpool = ctx.enter_context(tc.tile_pool(name="xpool", bufs=len(XT_PARTS)))
        w1pool = ctx.enter_context(tc.tile_pool(name="w1pool", bufs=len(W1_PARTS)))
        hpool = ctx.enter_context(tc.tile_pool(name="hpool", bufs=1))
        w2pool = ctx.enter_context(tc.tile_pool(name="w2pool", bufs=NJ))
        eppool = ctx.enter_context(tc.tile_pool(name="eppool", bufs=4))
        outpool = ctx.enter_context(tc.tile_pool(name="outpool", bufs=NJ))
        pspool = ctx.enter_context(tc.tile_pool(name="pspool", bufs=8, space="PSUM"))

        # --- constants (no-DMA first: feed the PE warm-up burst) ---
        ones_row = const.tile([128, 2, 256], F8, name="ones_row")
        nc.vector.memset(ones_row[:], 1.0)
        ones128 = const.tile([128, 2, 128], F8, name="ones128")
        nc.vector.memset(ones128[:], 1.0)
        nc1_sb = const.tile([128, 1], F32, name="nc1_sb")
        nc.vector.memset(nc1_sb[:], -C1)
        zero_sb = const.tile([128, 1], F32, name="zero_sb")
        nc.vector.memset(zero_sb[:], 0.0)

        # Dummy sigmoid as the FIRST ACT instruction: pins the one table
        # set (sigmoid_and_others: sigmoid + erf + parametric_relu) so the
        # whole kernel needs a single ACT_TABLE_LOAD.
        dum = const.tile([128, 1], F32, name="dum")
        nc.scalar.activation(dum[:], zero_sb[:], AF.Sigmoid, bias=zero_sb[:])

        # --- the full DMA schedule, issued up front, no reuse deps ---
        # ALL input streams ride the ONE SWDGE queue (gpsimd) in exact need
        # order: a single queue with >=2KB-per-partition packets sustains
        # ~420-450GB/s (the 16 shared DMA engines' cap), while splitting
        # across HWDGE+SWDGE queues measured ~25% LOWER aggregate.  Each
        # issue costs ~0.6us of gpsimd, well under per-chunk transfer time.
        xt_tiles = {}
        w1_tiles = []
        xq, wg = 0, 0
        xk = wk = 0
        while xk < KP1 or wk < KP1:
            # keep xT one pair-batch ahead of W1 in the queue
            if xq < len(XT_PARTS) and xk <= wk:
                npair = XT_PARTS[xq]
                xt_q = xpool.tile(
                    [128, max(XT_PARTS), 2, BS], F8, tag="xt", name=f"xt_q{xq}"
                )
                nc.gpsimd.dma_start(out=xt_q[:, :npair], in_=xT[:, xk : xk + npair])
                for i in range(npair):
                    xt_tiles[xk + i] = xt_q[:, i, :, :]
                xk += npair
                xq += 1
            else:
                npair = W1_PARTS[wg]
                w1g = w1pool.tile([128, 2, 2, H], F8, tag="w1t", name=f"w1g{wg}")
                nc.gpsimd.dma_start(out=w1g[:, :npair], in_=w1[:, wk : wk + npair])
                w1_tiles.append((wk, npair, w1g))
                wk += npair
                wg += 1
        assert xk == KP1 and wk == KP1

        # w2 chunks + merged consts + mun on the SWDGE queue, behind xT.
        # mun ships in two pieces so j0/j1's a2 (and thereby their psA
        # release chain) doesn't wait on the full 2.1MB transfer.
        mun_sb = const.tile([128, NJ, 2, 512], F16, name="mun_sb")
        w2_tiles = []
        cst8_sb = None
        cstf_sb = None
        for j in range(NJ):
            w2t = w2pool.tile(
                [128, KP2, 2, 2, 512], F8, tag="w2", name=f"w2t{j}"
            )
            nc.gpsimd.dma_start(out=w2t[:], in_=w2[:, j])
            w2_tiles.append(w2t)
            if j == 0:
                # t-row + W1 t-row (fp8) on one partition, plane 1 zeroed
                cst8_sb = const.tile([1, 2, BS + H], F8, name="cst8_sb")
                nc.vector.memset(cst8_sb[:, 1, :], 0.0)
                nc.gpsimd.dma_start(out=cst8_sb[:, 0, :], in_=cst8[:])
                # b1 columns + qm + qa (f32)
                cstf_sb = const.tile([128, KC2 + 4], F32, name="cstf_sb")
                nc.gpsimd.dma_start(out=cstf_sb[:], in_=cstf[:])
                nc.gpsimd.dma_start(out=mun_sb[:, :2], in_=mun[:, :2])
            if j == 2:
                # after w2 j2: j2's matmuls need that chunk before mun's rest
                nc.gpsimd.dma_start(out=mun_sb[:, 2:], in_=mun[:, 2:])
        tq_sb = cst8_sb[:, :, :BS]
        w1r_sb = cst8_sb[:, :, BS:]
        b1_sb = cstf_sb[:, :KC2]
        qm_sb = cstf_sb[:, KC2 : KC2 + 2]
        qa_sb = cstf_sb[:, KC2 + 2 :]

        # --- matmul1: h^T = W1^T @ x^T (fp8 DoubleRow, 17 pairs) ---
        ps1 = [
            pspool.tile([128, 512], F32, tag="ps", name=f"ps1_{m}")[:, :BS]
            for m in range(KC2)
        ]
        # PE warm-up on ps1[7]'s bank (its real k=0 matmul simply overwrites
        # via start=True): starts the HAM clock ramp without a 9th bank.
        for _ in range(WARM_N):
            nc.tensor.matmul(
                ps1[KC2 - 1], ones128[:], ones_row[:], start=True, stop=True,
                perf_mode=DR,
            )
        for k0, npair, w1g in w1_tiles:
            for kk in range(npair):
                k = k0 + kk
                rhs = xt_tiles[k]
                for m in range(KC2):
                    nc.tensor.matmul(
                        ps1[m],
                        w1g[:, kk, :, m * 128 : (m + 1) * 128],
                        rhs,
                        start=(k == 0),
                        stop=False,
                        perf_mode=DR,
                    )
        # t-row closes each accumulation group: K=1 DR (plane 1 zeroed)
        for m in range(KC2):
            nc.tensor.matmul(
                ps1[m],
                w1r_sb[:, :, m * 128 : (m + 1) * 128],
                tq_sb[:],
                start=False,
                stop=True,
                perf_mode=DR,
            )

        # h in fp8, pair-plane tiles for mm2's stationary operand:
        # h8 = SH * lrelu(x@W1 + b1) = Prelu(ps1/(SX*SW1/SH) + SH*b1)
        hp = []
        for kq in range(KP2):
            hp.append(hpool.tile([128, 2, BS], F8, tag=f"hp{kq}", name=f"hp{kq}"))
        for m in range(KC2):
            nc.scalar.activation(
                hp[m // 2][:, m % 2, :],
                ps1[m],
                AF.Prelu,
                bias=b1_sb[:, m : m + 1],
                scale=SH / (SX * SW1),
                alpha=ALPHA,
            )

        # --- matmul2 (fp8 DoubleRow) + fused epilogue ---
        pend = []

        def flush(item, halved):
            ji, g2s = item
            o2 = outpool.tile([128, 2, 512], F16, tag="o", name=f"O{ji}")
            handles = []
            if halved:
                # final j: per-batch-half Erf->DMA chain so the last out
                # write is half-size and its issue overlaps the first Erf
                for bh in range(2):
                    a = nc.scalar.activation(
                        o2[:, bh, :], g2s[bh][:], AF.Erf, bias=zero_sb[:]
                    )
                    d = nc.sync.dma_start(
                        out=outd[:, ji, bh : bh + 1, :], in_=o2[:, bh : bh + 1, :]
                    )
                    handles += [a, d]
            else:
                for bh in range(2):
                    handles.append(
                        nc.scalar.activation(
                            o2[:, bh, :], g2s[bh][:], AF.Erf, bias=zero_sb[:]
                        )
                    )
                handles.append(nc.sync.dma_start(out=outd[:, ji], in_=o2[:]))
            return handles

        for j in range(NJ):
            # psB allocated first: the B-side MMs run first, so they get
            # the banks released earliest
            psB = [
                pspool.tile([128, 512], F32, tag="ps", name=f"psB{j}_{bh}")
                for bh in range(2)
            ]
            psA = [
                pspool.tile([128, 512], F32, tag="ps", name=f"psA{j}_{bh}")
                for bh in range(2)
            ]
            w2t = w2_tiles[j]
            # a2 = mu*qm + qa on gpsimd (idle after its DMA issues): keeps
            # DVE under mm2's streaming rate
            a2s = []
            for bh in range(2):
                a2 = eppool.tile([128, 512], F32, tag="A", name=f"A{j}_{bh}", bufs=4)
                nc.gpsimd.tensor_scalar(
                    a2[:],
                    mun_sb[:, j, bh, :],
                    qm_sb[:, bh : bh + 1],
                    qa_sb[:, bh : bh + 1],
                    OP.mult,
                    OP.add,
                )
                a2s.append(a2)
            # All B-side MMs first: psB stops 8 MMs before the block end,
            # so the Sigmoids (which release psB banks and head the final
            # tail chain) overlap the A-side MMs.
            for half, ps in ((1, psB), (0, psA)):
                for kq in range(KP2):
                    for bh in range(2):
                        lhsT = hp[kq][:, :, bh * 128 : (bh + 1) * 128]
                        nc.tensor.matmul(
                            ps[bh][:], lhsT, w2t[:, kq, :, half, :],
                            start=(kq == 0), stop=(kq == KP2 - 1), perf_mode=DR,
                        )
            # Consume psB first (Sigmoid on ACT), then psA (DVE add) so the
            # banks release early for j+2.  E/s2/g2 in bf16: full fp32 range
            # (|psA + a2| can exceed fp16 max) at 16-bit DVE rates.
            # Sigmoids enter the ACT FIFO BEFORE j-1's Erfs: psB's release
            # is what unblocks j+2's first matmuls, the Erfs aren't critical.
            # Exception: the LAST j's previous flush goes first, so the
            # final tail chain isn't queued behind a long-waiting Sigmoid.
            if j == NJ - 1 and pend:
                flush(pend.pop(0), halved=False)
            e2s = []
            for bh in range(2):
                e2 = eppool.tile([128, 512], BF16, tag="E", name=f"E{j}_{bh}", bufs=4)
                nc.scalar.activation(
                    e2[:], psB[bh][:], AF.Sigmoid, bias=nc1_sb[:],
                    scale=-1.0 / SCALE2,
                )
                e2s.append(e2)
            if pend:
                flush(pend.pop(0), halved=False)
            g2s = []
            for bh in range(2):
                s2 = eppool.tile([128, 512], BF16, tag="S", name=f"S{j}_{bh}", bufs=4)
                nc.vector.tensor_tensor(s2[:], psA[bh][:], a2s[bh][:], OP.add)
                g2 = eppool.tile([128, 512], BF16, tag="G", name=f"G{j}_{bh}", bufs=4)
                nc.vector.tensor_tensor(g2[:], s2[:], e2s[bh][:], OP.mult)
                g2s.append(g2)
            pend.append((j, g2s))
        tail_handles = flush(pend.pop(0), halved=True)
        assert not pend

        # PE clock keep-alive: HAM downclocks the Tensor engine ~4us after
        # its last matmul, and the compiler's end-of-kernel semaphore-clear
        # storm then runs at HALF dispatch rate (~6.5us on Tensor).  A few
        # tiny matmuls pinned to the tail epilogue's instructions keep the
        # PE "active" through the final drain so the clears run full-speed.
        ps_keep = pspool.tile([128, 512], F32, tag="ps", name="ps_keep")
        for i, dep in enumerate(tail_handles):
            mm = nc.tensor.matmul(
                ps_keep[:, :64], ones128[:], ones_row[:, :, :64],
                start=True, stop=True, perf_mode=DR,
            )
            add_dep_helper(mm.ins, dep.ins, True, "PE clock keepalive")


_NC = None
_last_in_maps = None


def kernel(mu, t, gamma, W1, b1, W2, b2):
    global _NC
    if _NC is None:
        _NC = _build()
    nc = _NC

    import ml_dtypes

    E4 = ml_dtypes.float8_e4m3
    f16 = np.float16
    f32 = np.float32

    # x^T rows 0..4095 (mu) in partition-major DR pair layout; the t
    # feature row ships in cst8 for the K=1 closers
    X8 = (np.asarray(mu, dtype=f32).T * SX).astype(E4)          # [D, B]
    t8 = (np.asarray(t, dtype=f32)[:, 0] * SX).astype(E4)

    W1q = (np.asarray(W1, dtype=f32) * SW1).astype(E4)
    W1q[:, H - 1] = 0.0
    # [k_pair, plane, p, h] -> [p, k_pair, plane, h]
    w1_np = np.ascontiguousarray(
        W1q[:D].reshape(KP1, 2, 128, H).transpose(2, 0, 1, 3)
    )
    w1r_np = np.ascontiguousarray(W1q[D])                        # [H]
    W2n = np.asarray(W2, dtype=f32).copy()
    W2n[H - 1, :] = np.asarray(b2, dtype=f32)
    w2_np = (W2n * SW2).astype(E4)
    w2_np = np.ascontiguousarray(
        w2_np.reshape(KP2, 2, 128, 2, NJ, 512).transpose(2, 4, 0, 1, 3, 5)
    )
    b1n = np.asarray(b1, dtype=f32).copy()
    b1n[H - 1] = 1.0
    b1c_np = (b1n * SH).reshape(KC2, 128).T                      # [128, KC2]

    g64 = np.asarray(gamma, dtype=np.float64)[:, 0]
    s64 = np.sqrt((1.0 - g64) / g64)
    qm_full = (-SCALE2 / (g64 * s64)).astype(f32)
    qa_full = (SCALE2 * 0.875 / s64).astype(f32)
    mu16 = np.asarray(mu, dtype=f16)

    in_maps = []
    for c in range(NCORES):
        sl = slice(c * BS, (c + 1) * BS)
        # [k_pair, plane, p, b] -> [p, k_pair, plane, b]
        xt_c = np.ascontiguousarray(
            X8[:, sl].reshape(KP1, 2, 128, BS).transpose(2, 0, 1, 3)
        )
        # [bh, p, j, c] -> [p, j, bh, c]
        mun_c = np.ascontiguousarray(
            mu16[sl].reshape(2, 128, NJ, 512).transpose(1, 2, 0, 3)
        )
        cst8_c = np.concatenate([t8[sl], w1r_np]).reshape(1, BS + H)
        cstf_c = np.ascontiguousarray(
            np.concatenate(
                [
                    b1c_np,
                    qm_full[sl].reshape(2, 128).T,
                    qa_full[sl].reshape(2, 128).T,
                ],
                axis=1,
            )
        )
        in_maps.append(
            {
                "xT": xt_c,
                "w1": w1_np,
                "w2": w2_np,
                "mun": mun_c,
                "cst8": np.ascontiguousarray(cst8_c),
                "cstf": cstf_c,
            }
        )

    global _last_in_maps
    _last_in_maps = in_maps

    res = run_bass_kernel_spmd(nc, in_maps, core_ids=list(range(NCORES)))
    # out[p, j, bh, c] -> rows bh*128+p, cols j*512+c
    return np.concatenate(
        [
            np.asarray(r["out"])
            .transpose(2, 0, 1, 3)
            .reshape(BS, D)
            .astype(np.float32)
            * 0.5
            + 0.5
            for r in res.results
        ],
        axis=0,
    )


# revision 28
# speedup vs baseline: 1.0673x; 1.0673x over previous
"""Trainium2 Bass kernel for nn_DiscretisedBNF (histogram binning MLP).

Math: the reference's per-bin CDF sum telescopes exactly, so

    out = 0.5*(1 + erf(arg)),  arg = (0.875 - mu_x) * inv
    mu_x = mu/gamma - s*mu_eps,  inv = exp(-ln_sigma_eps)/(s*sqrt(2))

Rearranged for the chip with fp8 scaling (SX*SW1 = 2048 for mm1,
SH*SW2 = 1024 for mm2; PSUM carries 1024x the true nn outputs):

    arg = (psA + mu*qm + qa) * E
    qm  = -1024/(gamma*s), qa = 896/s            (per batch row, fp32)
    E   = exp(-psB/1024 - ln(1024*sqrt(2)))  ~=  sigmoid(-psB/1024 - C1)
    out_dram = erf(arg)      (host applies 0.5*x + 0.5)

The sigmoid stand-in for exp keeps every activation (Prelu for the MLP's
leaky relu, Sigmoid, Erf) inside the single `sigmoid_and_others` ACT
table set -- one table load for the whole kernel.

Both matmuls run fp8e4 DoubleRow (2 weights/cell).  The b2 bias rides on
hidden unit 1023: W1 col 1023 is zeroed, b1[1023] forced to 1, and W2
row 1023 replaced by b2.

DMA schedule: every DRAM tensor is PARTITION-MAJOR (2-16KB contiguous
per partition -> ~26GB/s per DMA engine, ~420GB/s aggregate over the 16
shared engines).  Everything is fully SBUF-resident, issued up front in
need order -- DMA issue on an engine costs ~0.6us per instruction, so
xT leads the SWDGE queue and the small constants are merged into two
tensors and issued behind the first w2 chunk:
  - gpsimd SWDGE:   xT p0/p1/p2, w2 j0, consts, w2 j1, mun, w2 j2..j7
  - sync/scalar HWDGE: W1 in 9 groups (1,1,2,...) alternating rings
  - sync HWDGE:     out writes as each j-group finishes
a2 = mu*qm+qa runs on gpsimd (idle after its DMA issues) to keep DVE
under mm2's 216ns/matmul streaming rate.
Sharding: pure data parallel -- batch dim (2048) split 256 rows/core.
"""

import numpy as np
from contextlib import ExitStack

import concourse.bass as bass
import concourse.mybir as mybir
from concourse.tile import TileContext
from concourse.tile_rust import add_dep_helper
from concourse.bass_utils import run_bass_kernel_spmd

B, D, H = 2048, 4096, 1024
NCORES = 8
BS = B // NCORES            # 256 batch rows per core
KP1 = 16                    # mm1 DoubleRow pairs over mu rows (4096)
W1_PARTS = [1, 1, 2, 2, 2, 2, 2, 2, 2]   # 9 W1 groups (pairs per group)
XT_PARTS = [1, 3, 6, 6]                  # xT delivery parts (pairs)
KC2 = H // 128              # 8 h chunks of 128
KP2 = KC2 // 2              # 4 mm2 DoubleRow pairs
NJ = D // 512               # 8 output column groups of 512
ALPHA = 0.01                # torch nn.LeakyReLU default
SX, SW1, SH, SW2 = 16.0, 128.0, 16.0, 64.0
SCALE2 = SH * SW2           # 1024: psum = SCALE2 * (h @ W2 + b2)
C1 = float(np.log(SCALE2 * np.sqrt(2.0)))
# 10 warmups fit inside the DMA ramp window (free PE time) and open the
# HAM clock gate ~2.5us earlier; 20 overdraw the power budget and cause
# a mid-mm2 K=4 throttle window (measured).
WARM_N = 10

F8 = mybir.dt.float8e4
F16 = mybir.dt.float16
BF16 = mybir.dt.bfloat16
F32 = mybir.dt.float32
AF = mybir.ActivationFunctionType
OP = mybir.AluOpType
DR = mybir.MatmulPerfMode.DoubleRow


def split_multi_waits(nc):
    """This container's walrus accepts at most ONE sync-wait per instruction
    (setupSyncWait: 'Too many sync wait commands').  Split any instruction
    carrying N>1 waits into N-1 single-wait NoOps on the same engine placed
    immediately before it."""
    cnt = 0
    sync_info_cls = None
    for f in nc.m.functions:
        for bb in f.blocks:
            out = []
            changed = False
            for inst in bb.instructions:
                si = inst.sync_info
                waits = list(si.on_wait) if si and si.on_wait else []
                if len(waits) > 1:
                    if sync_info_cls is None:
                        sync_info_cls = type(si)
                    for w in waits[:-1]:
                        nop = mybir.InstNoOp(name=f"waitsplit_{cnt}", ins=[], outs=[])
                        cnt += 1
                        nop.engine = inst.engine
                        nop.sync_info = sync_info_cls(on_wait=[w], on_update=[])
                        out.append(nop)
                    si.on_wait = waits[-1:]
                    changed = True
                out.append(inst)
            if changed:
                bb.instructions = out
    return cnt


def _lean_drain_and_barrier(self, tick_clock, wait_clock):
    """Replacement for TileContext._drain_and_barrier: drain + ONE barrier,
    skipping the semaphore-clear butterfly (the Bass preamble re-clears all
    kernel semaphores at the start of each execution anyway)."""
    import concourse.tile as tile_mod

    nc = self.nc
    drain_inst = nc.sync.drain()
    wait_clock.add_sem_waits(
        drain_inst.ins, tile_mod.ScopedClock({None: tick_clock.global_clock})
    )
    popped = nc._tile_sem_poison_stack.pop()
    assert popped is self._sem_poison


def _build():
    nc = bass.Bass()
    orig_drain = TileContext._drain_and_barrier
    TileContext._drain_and_barrier = _lean_drain_and_barrier
    try:
        _build_body(nc)
    finally:
        TileContext._drain_and_barrier = orig_drain

    split_multi_waits(nc)
    return nc


def _build_body(nc):
    # all partition-major: per-partition lines are contiguous and big
    xT = nc.dram_tensor("xT", [128, KP1, 2, BS], F8, kind="ExternalInput")
    w1 = nc.dram_tensor("w1", [128, KP1, 2, H], F8, kind="ExternalInput")
    w2 = nc.dram_tensor("w2", [128, NJ, KP2, 2, 2, 512], F8, kind="ExternalInput")
    mun = nc.dram_tensor("mun", [128, NJ, 2, 512], F16, kind="ExternalInput")
    cst8 = nc.dram_tensor("cst8", [1, BS + H], F8, kind="ExternalInput")
    cstf = nc.dram_tensor("cstf", [128, KC2 + 4], F32, kind="ExternalInput")
    outd = nc.dram_tensor("out", [128, NJ, 2, 512], F16, kind="ExternalOutput")

    with TileContext(nc) as tc, ExitStack() as ctx:
        const = ctx.enter_context(tc.tile_pool(name="const", bufs=1))
        xpool = ctx.enter_context(tc.tile_pool(name="xpool", bufs=len(XT_PARTS)))
        w1pool = ctx.enter_context(tc.tile_pool(name="w1pool", bufs=len(W1_PARTS)))
        hpool = ctx.enter_context(tc.tile_pool(name="hpool", bufs=1))
        w2pool = ctx.enter_context(tc.tile_pool(name="w2pool", bufs=NJ))
        eppool = ctx.enter_context(tc.tile_pool(name="eppool", bufs=4))
        outpool = ctx.enter_context(tc.tile_pool(name="outpool", bufs=NJ))
        pspool = ctx.enter_context(tc.tile_pool(name="pspool", bufs=8, space="PSUM"))

        # --- constants (no-DMA first: feed the PE warm-up burst) ---
        ones_row = const.tile([128, 2, 256], F8, name="ones_row")
        nc.vector.memset(ones_row[:], 1.0)
        ones128 = const.tile([128, 2, 128], F8, name="ones128")
        nc.vector.memset(ones128[:], 1.0)
        nc1_sb = const.tile([128, 1], F32, name="nc1_sb")
        nc.vector.memset(nc1_sb[:], -C1)
        zero_sb = const.tile([128, 1], F32, name="zero_sb")
        nc.vector.memset(zero_sb[:], 0.0)

        # Dummy sigmoid as the FIRST ACT instruction: pins the one table
        # set (sigmoid_and_others: sigmoid + erf + parametric_relu) so the
        # whole kernel needs a single ACT_TABLE_LOAD.
        dum = const.tile([128, 1], F32, name="dum")
        nc.scalar.activation(dum[:], zero_sb[:], AF.Sigmoid, bias=zero_sb[:])

        # --- the full DMA schedule, issued up front, no reuse deps ---
        # ALL input streams ride the ONE SWDGE queue (gpsimd) in exact need
        # order: a single queue with >=2KB-per-partition packets sustains
        # ~420-450GB/s (the 16 shared DMA engines' cap), while splitting
        # across HWDGE+SWDGE queues measured ~25% LOWER aggregate.  Each
        # issue costs ~0.6us of gpsimd, well under per-chunk transfer time.
        xt_tiles = {}
        w1_tiles = []
        xq, wg = 0, 0
        xk = wk = 0
        while xk < KP1 or wk < KP1:
            # keep xT one pair-batch ahead of W1 in the queue
            if xq < len(XT_PARTS) and xk <= wk:
                npair = XT_PARTS[xq]
                xt_q = xpool.tile(
                    [128, max(XT_PARTS), 2, BS], F8, tag="xt", name=f"xt_q{xq}"
                )
                nc.gpsimd.dma_start(out=xt_q[:, :npair], in_=xT[:, xk : xk + npair])
                for i in range(npair):
                    xt_tiles[xk + i] = xt_q[:, i, :, :]
                xk += npair
                xq += 1
            else:
                npair = W1_PARTS[wg]
                w1g = w1pool.tile([128, 2, 2, H], F8, tag="w1t", name=f"w1g{wg}")
                nc.gpsimd.dma_start(out=w1g[:, :npair], in_=w1[:, wk : wk + npair])
                w1_tiles.append((wk, npair, w1g))
                wk += npair
                wg += 1
        assert xk == KP1 and wk == KP1

        # w2 chunks + merged consts + mun on the SWDGE queue, behind xT.
        # mun ships in two pieces so j0/j1's a2 (and thereby their psA
        # release chain) doesn't wait on the full 2.1MB transfer.
        mun_sb = const.tile([128, NJ, 2, 512], F16, name="mun_sb")
        w2_tiles = []
        cst8_sb = None
        cstf_sb = None
        for j in range(NJ):
            w2t = w2pool.tile(
                [128, KP2, 2, 2, 512], F8, tag="w2", name=f"w2t{j}"
            )
            nc.gpsimd.dma_start(out=w2t[:], in_=w2[:, j])
            w2_tiles.append(w2t)
            if j == 0:
                # t-row + W1 t-row (fp8) on one partition, plane 1 zeroed
                cst8_sb = const.tile([1, 2, BS + H], F8, name="cst8_sb")
                nc.vector.memset(cst8_sb[:, 1, :], 0.0)
                nc.gpsimd.dma_start(out=cst8_sb[:, 0, :], in_=cst8[:])
                # b1 columns + qm + qa (f32)
                cstf_sb = const.tile([128, KC2 + 4], F32, name="cstf_sb")
                nc.gpsimd.dma_start(out=cstf_sb[:], in_=cstf[:])
                nc.gpsimd.dma_start(out=mun_sb[:, :2], in_=mun[:, :2])
            if j == 2:
                # after w2 j2: j2's matmuls need that chunk before mun's rest
                nc.gpsimd.dma_start(out=mun_sb[:, 2:], in_=mun[:, 2:])
        tq_sb = cst8_sb[:, :, :BS]
        w1r_sb = cst8_sb[:, :, BS:]
        b1_sb = cstf_sb[:, :KC2]
        qm_sb = cstf_sb[:, KC2 : KC2 + 2]
        qa_sb = cstf_sb[:, KC2 + 2 :]

        # --- matmul1: h^T = W1^T @ x^T (fp8 DoubleRow, 17 pairs) ---
        ps1 = [
            pspool.tile([128, 512], F32, tag="ps", name=f"ps1_{m}")[:, :BS]
            for m in range(KC2)
        ]
        # PE warm-up on ps1[7]'s bank (its real k=0 matmul simply overwrites
        # via start=True): starts the HAM clock ramp without a 9th bank.
        for _ in range(WARM_N):
            nc.tensor.matmul(
                ps1[KC2 - 1], ones128[:], ones_row[:], start=True, stop=True,
                perf_mode=DR,
            )
        for k0, npair, w1g in w1_tiles:
            for kk in range(npair):
                k = k0 + kk
                rhs = xt_tiles[k]
                for m in range(KC2):
                    nc.tensor.matmul(
                        ps1[m],
                        w1g[:, kk, :, m * 128 : (m + 1) * 128],
                        rhs,
                        start=(k == 0),
                        stop=False,
                        perf_mode=DR,
                    )
        # t-row closes each accumulation group: K=1 DR (plane 1 zeroed)
        for m in range(KC2):
            nc.tensor.matmul(
                ps1[m],
                w1r_sb[:, :, m * 128 : (m + 1) * 128],
                tq_sb[:],
                start=False,
                stop=True,
                perf_mode=DR,
            )

        # h in fp8, pair-plane tiles for mm2's stationary operand:
        # h8 = SH * lrelu(x@W1 + b1) = Prelu(ps1/(SX*SW1/SH) + SH*b1)
        hp = []
        for kq in range(KP2):
            hp.append(hpool.tile([128, 2, BS], F8, tag=f"hp{kq}", name=f"hp{kq}"))
        for m in range(KC2):
            nc.scalar.activation(
                hp[m // 2][:, m % 2, :],
                ps1[m],
                AF.Prelu,
                bias=b1_sb[:, m : m + 1],
                scale=SH / (SX * SW1),
                alpha=ALPHA,
            )

        # --- matmul2 (fp8 DoubleRow) + fused epilogue ---
        pend = []

        def flush(item, halved):
            ji, g2s = item
            o2 = outpool.tile([128, 2, 512], F16, tag="o", name=f"O{ji}")
            handles = []
            if halved:
                # final j: per-batch-half Erf->DMA chain so the last out
                # write is half-size and its issue overlaps the first Erf
                for bh in range(2):
                    a = nc.scalar.activation(
                        o2[:, bh, :], g2s[bh][:], AF.Erf, bias=zero_sb[:]
                    )
                    d = nc.sync.dma_start(
                        out=outd[:, ji, bh : bh + 1, :], in_=o2[:, bh : bh + 1, :]
                    )
                    handles += [a, d]
            else:
                for bh in range(2):
                    handles.append(
                        nc.scalar.activation(
                            o2[:, bh, :], g2s[bh][:], AF.Erf, bias=zero_sb[:]
                        )
                    )
                handles.append(nc.sync.dma_start(out=outd[:, ji], in_=o2[:]))
            return handles

        for j in range(NJ):
            # psB allocated first: the B-side MMs run first, so they get
            # the banks released earliest
            psB = [
                pspool.tile([128, 512], F32, tag="ps", name=f"psB{j}_{bh}")
                for bh in range(2)
            ]
            psA = [
                pspool.tile([128, 512], F32, tag="ps", name=f"psA{j}_{bh}")
                for bh in range(2)
            ]
            w2t = w2_tiles[j]
            # a2 = mu*qm + qa on gpsimd (idle after its DMA issues): keeps
            # DVE under mm2's streaming rate
            a2s = []
            for bh in range(2):
                a2 = eppool.tile([128, 512], F32, tag="A", name=f"A{j}_{bh}", bufs=4)
                nc.gpsimd.tensor_scalar(
                    a2[:],
                    mun_sb[:, j, bh, :],
                    qm_sb[:, bh : bh + 1],
                    qa_sb[:, bh : bh + 1],
                    OP.mult,
                    OP.add,
                )
                a2s.append(a2)
            # All B-side MMs first: psB stops 8 MMs before the block end,
            # so the Sigmoids (which release psB banks and head the final
            # tail chain) overlap the A-side MMs.
            for half, ps in ((1, psB), (0, psA)):
                for kq in range(KP2):
                    for bh in range(2):
                        lhsT = hp[kq][:, :, bh * 128 : (bh + 1) * 128]
                        nc.tensor.matmul(
                            ps[bh][:], lhsT, w2t[:, kq, :, half, :],
                            start=(kq == 0), stop=(kq == KP2 - 1), perf_mode=DR,
                        )
            # Consume psB first (Sigmoid on ACT), then psA (DVE add) so the
            # banks release early for j+2.  E/s2/g2 in bf16: full fp32 range
            # (|psA + a2| can exceed fp16 max) at 16-bit DVE rates.
            # Sigmoids enter the ACT FIFO BEFORE j-1's Erfs: psB's release
            # is what unblocks j+2's first matmuls, the Erfs aren't critical.
            e2s = []
            for bh in range(2):
                e2 = eppool.tile([128, 512], BF16, tag="E", name=f"E{j}_{bh}", bufs=4)
                nc.scalar.activation(
                    e2[:], psB[bh][:], AF.Sigmoid, bias=nc1_sb[:],
                    scale=-1.0 / SCALE2,
                )
                e2s.append(e2)
            if pend:
                flush(pend.pop(0), halved=False)
            g2s = []
            for bh in range(2):
                s2 = eppool.tile([128, 512], BF16, tag="S", name=f"S{j}_{bh}", bufs=4)
                nc.vector.tensor_tensor(s2[:], psA[bh][:], a2s[bh][:], OP.add)
                g2 = eppool.tile([128, 512], BF16, tag="G", name=f"G{j}_{bh}", bufs=4)
                nc.vector.tensor_tensor(g2[:], s2[:], e2s[bh][:], OP.mult)
                g2s.append(g2)
            pend.append((j, g2s))
        flush(pend.pop(0), halved=True)
        assert not pend


_NC = None
_last_in_maps = None


def kernel(mu, t, gamma, W1, b1, W2, b2):
    global _NC
    if _NC is None:
        _NC = _build()
    nc = _NC

    import ml_dtypes

    E4 = ml_dtypes.float8_e4m3
    f16 = np.float16
    f32 = np.float32

    # x^T rows 0..4095 (mu) in partition-major DR pair layout; the t
    # feature row ships in cst8 for the K=1 closers
    X8 = (np.asarray(mu, dtype=f32).T * SX).astype(E4)          # [D, B]
    t8 = (np.asarray(t, dtype=f32)[:, 0] * SX).astype(E4)

    W1q = (np.asarray(W1, dtype=f32) * SW1).astype(E4)
    W1q[:, H - 1] = 0.0
    # [k_pair, plane, p, h] -> [p, k_pair, plane, h]
    w1_np = np.ascontiguousarray(
        W1q[:D].reshape(KP1, 2, 128, H).transpose(2, 0, 1, 3)
    )
    w1r_np = np.ascontiguousarray(W1q[D])                        # [H]
    W2n = np.asarray(W2, dtype=f32).copy()
    W2n[H - 1, :] = np.asarray(b2, dtype=f32)
    w2_np = (W2n * SW2).astype(E4)
    w2_np = np.ascontiguousarray(
        w2_np.reshape(KP2, 2, 128, 2, NJ, 512).transpose(2, 4, 0, 1, 3, 5)
    )
    b1n = np.asarray(b1, dtype=f32).copy()
    b1n[H - 1] = 1.0
    b1c_np = (b1n * SH).reshape(KC2, 128).T                      # [128, KC2]

    g64 = np.asarray(gamma, dtype=np.float64)[:, 0]
    s64 = np.sqrt((1.0 - g64) / g64)
    qm_full = (-SCALE2 / (g64 * s64)).astype(f32)
    qa_full = (SCALE2 * 0.875 / s64).astype(f32)
    mu16 = np.asarray(mu, dtype=f16)

    in_maps = []
    for c in range(NCORES):
        sl = slice(c * BS, (c + 1) * BS)
        # [k_pair, plane, p, b] -> [p, k_pair, plane, b]
        xt_c = np.ascontiguousarray(
            X8[:, sl].reshape(KP1, 2, 128, BS).transpose(2, 0, 1, 3)
        )
        # [bh, p, j, c] -> [p, j, bh, c]
        mun_c = np.ascontiguousarray(
            mu16[sl].reshape(2, 128, NJ, 512).transpose(1, 2, 0, 3)
        )
        cst8_c = np.concatenate([t8[sl], w1r_np]).reshape(1, BS + H)
        cstf_c = np.ascontiguousarray(
            np.concatenate(
                [
                    b1c_np,
                    qm_full[sl].reshape(2, 128).T,
                    qa_full[sl].reshape(2, 128).T,
                ],
                axis=1,
            )
        )
        in_maps.append(
            {
                "xT": xt_c,
                "w1": w1_np,
                "w2": w2_np,
                "mun": mun_c,
                "cst8": np.ascontiguousarray(cst8_c),
                "cstf": cstf_c,
            }
        )

    global _last_in_maps
    _last_in_maps = in_maps

    res = run_bass_kernel_spmd(nc, in_maps, core_ids=list(range(NCORES)))
    # out[p, j, bh, c] -> rows bh*128+p, cols j*512+c
    return np.concatenate(
        [
            np.asarray(r["out"])
            .transpose(2, 0, 1, 3)
            .reshape(BS, D)
            .astype(np.float32)
            * 0.5
            + 0.5
            for r in res.results
        ],
        axis=0,
    )
